# revision 1
# baseline (speedup 1.0000x reference)
"""Trainium2 Bass kernel for MAGNN link prediction (nn_MAGNN_lp).

Sharding: the B=8192 link-prediction targets are sharded across 8 cores
(1024 each); metapath instances are sharded by target range, so the segment
softmax/segment-sum is entirely core-local. The node towers are sharded by
node rows (5000/core) and the projected [40000,64] node table is AllGathered
in DRAM. Per metapath each core dma_gathers its instances' node vectors
(int16 gather indices force a lo/hi table split; instances are grouped into
class-pure tile segments by which halves their 3 nodes fall in), applies the
(identity, rot, identity) cumulative complex rotation (positions 0/2 are
identity because the normalized rotation times its conjugate is 1), computes
attention logits, exponentiates (softmax without max-subtraction; logits are
O(1)), and dumps rows of [exp(e)*eft | exp(e)] densely to DRAM. The head
dma_gathers each target's instance rows (host-built index lists; targets are
permuted by descending instance count per core so the per-b-tile gather
width shrinks), reduces them, normalizes, applies ELU, runs semantic
attention (AllReduce of 4 partial sums), the product MLP, and a 2-way
softmax; the host inverts the target permutation when assembling the output.
Host work is slicing/packing of index tensors only.
"""
import math
from dataclasses import dataclass

import numpy as np

import concourse.bass as bass
import concourse.mybir as mybir
import concourse.tile as tile
from concourse import bacc
from concourse.masks import make_identity

F32 = mybir.dt.float32
I32 = mybir.dt.int32
I16 = mybir.dt.int16
AF = mybir.ActivationFunctionType
ALU = mybir.AluOpType
PSUM = "PSUM"


@dataclass
class Cfg:
    n_cores: int = 8
    B: int = 8192
    HID: int = 64
    H: int = 8
    D: int = 8
    F0: int = 512
    AV: int = 128
    CH: int = 128
    n_nodes: int = 40000
    LO: int = 32768        # lo/hi table split (int16 gather index limit)
    T: int = 200           # 128-instance tiles per metapath per core
    Tc: int = 20           # tiles per processing chunk
    n_mp: int = 4
    gelu: bool = True      # False: Tanh stand-in (CoreSim lacks Gelu)
    debug: bool = False
    tiles_per_class: np.ndarray | None = None   # [n_mp, 8]
    KP: np.ndarray | None = None                # [n_mp, b_tiles]

    @property
    def B_loc(self):
        return self.B // self.n_cores

    @property
    def nodes_core(self):
        return self.n_nodes // self.n_cores

    @property
    def node_tiles(self):
        return (self.nodes_core + 127) // 128

    @property
    def E_loc(self):
        return self.T * 128

    @property
    def n_chunks(self):
        return self.T // self.Tc

    @property
    def kF(self):
        return self.F0 // 128

    @property
    def b_tiles(self):
        return self.B_loc // 128

    @property
    def KP_max(self):
        return int(self.KP.max())


def _ap_with(ap, offset_delta, tail_dims):
    """Copy an AP, keeping its partition dim, replacing trailing free dims."""
    return bass.AP(ap.tensor, ap.offset + offset_delta,
                   [list(ap.ap[0])] + [list(d) for d in tail_dims])


def _class_segments(tiles_per_class):
    segs, t = [], 0
    for cls in range(8):
        n = int(tiles_per_class[cls])
        if n:
            segs.append((cls, t, t + n))
            t += n
    return segs


def _gather_calls(tiles_per_class, Tc, n_chunks):
    """calls[ch][l] = [(toff_rel, ntiles, hi)], adjacent same-hi merged."""
    segs = _class_segments(tiles_per_class)
    calls = []
    for ch in range(n_chunks):
        c0, c1 = ch * Tc, (ch + 1) * Tc
        per_l = []
        for l in range(3):
            lst = []
            for cls, s0, s1 in segs:
                a, b = max(c0, s0), min(c1, s1)
                if a >= b:
                    continue
                hi = bool((cls >> l) & 1)
                if lst and lst[-1][2] == hi and lst[-1][0] + lst[-1][1] == a - c0:
                    lst[-1] = (lst[-1][0], lst[-1][1] + (b - a), hi)
                else:
                    lst.append((a - c0, b - a, hi))
            per_l.append(lst)
        calls.append(per_l)
    return calls


def build_program(cfg: Cfg):
    c = cfg
    assert c.tiles_per_class is not None and c.KP is not None
    nc = bacc.Bacc("TRN2", target_bir_lowering=False, debug=False,
                   num_devices=c.n_cores)

    def di(name, shape, dtype=F32):
        return nc.dram_tensor(name, list(shape), dtype, kind="ExternalInput")

    T8 = c.T * 8
    feats = di("feats", (c.node_tiles * 128, c.F0))
    pw = di("pw", (c.F0, c.HID))
    pb = di("pb", (c.HID,))
    w2 = di("w2", (c.HID, c.HID))
    b2 = di("b2", (c.HID,))
    g = di("g", (c.HID,))
    be = di("be", (c.HID,))
    rvec = di("rvec", (c.HID,))
    attn = di("attn", (c.n_mp, c.HID))
    emi16 = di("emi16", (c.n_mp * 3 * 128, T8), I16)
    gidx16 = di("gidx16", (c.n_mp * c.b_tiles * 128, c.KP_max * 8), I16)
    suw1 = di("suw1", (c.HID, c.AV))
    sub1 = di("sub1", (c.AV,))
    suw2 = di("suw2", (c.AV,))
    siw1 = di("siw1", (c.HID, c.AV))
    sib1 = di("sib1", (c.AV,))
    siw2 = di("siw2", (c.AV,))
    cw1 = di("cw1", (c.HID, c.CH))
    cb1 = di("cb1", (c.CH,))
    cw2 = di("cw2", (c.CH, 2))
    outd = nc.dram_tensor("out", [c.B_loc, 2], F32, kind="ExternalOutput")
    if c.debug:
        dbg_ed = nc.dram_tensor("dbg_ed", [128, 3 * c.Tc * c.HID], F32, kind="ExternalOutput")
        dbg_row = nc.dram_tensor("dbg_row", [128, c.Tc * 128], F32, kind="ExternalOutput")
        dbg_acc = nc.dram_tensor("dbg_acc", [128, 72], F32, kind="ExternalOutput")
        dbg_sem = nc.dram_tensor("dbg_sem", [1, 12], F32, kind="ExternalOutput")

    HID, H, D = c.HID, c.H, c.D
    NPAIR = HID // 2

    with tile.TileContext(nc) as tc:
        with (
            tc.tile_pool(name="const", bufs=1) as kpool,
            tc.tile_pool(name="dram", bufs=1, space="DRAM") as dpool,
        ):
            pk_ctx = tc.tile_pool(name="ps_const", bufs=1, space="PSUM")
            pkpool = pk_ctx.__enter__()
            # ---------- constants ----------
            id128 = kpool.tile([128, 128], F32, tag="id128")
            make_identity(nc, id128[:])
            ones1 = kpool.tile([1, 128], F32, tag="ones1")
            nc.vector.memset(ones1[:], 1.0)
            onescol = kpool.tile([128, 1], F32, tag="onescol")
            nc.vector.memset(onescol[:], 1.0)
            zerot = kpool.tile([128, 128], F32, tag="zerot")
            nc.vector.memset(zerot[:], 0.0)
            epscol = kpool.tile([128, 1], F32, tag="epscol")
            nc.vector.memset(epscol[:], 1e-5)

            def rep_row(dram_vec, n, scale=None, tag=None):
                row = kpool.tile([1, n], F32, tag=f"{tag}_row")
                nc.sync.dma_start(row[:], dram_vec)
                ps = pkpool.tile([128, 512], F32, space=PSUM, tag="reppsum")
                nc.tensor.matmul(out=ps[:, :n], lhsT=ones1[:], rhs=row[:],
                                 start=True, stop=True)
                rep = kpool.tile([128, n], F32, tag=tag)
                if scale is None:
                    nc.vector.tensor_copy(rep[:], ps[:, :n])
                else:
                    nc.vector.tensor_scalar_mul(rep[:], ps[:, :n], scale)
                return rep

            def vrow(x):
                return x.ap().rearrange("(o a) -> o a", o=1)

            PBrep = rep_row(vrow(pb), HID, tag="PBrep")
            B2rep = rep_row(vrow(b2), HID, tag="B2rep")
            G3rep = rep_row(vrow(g), HID, scale=1.0 / 3.0, tag="G3rep")
            BE3rep = rep_row(vrow(be), HID, scale=1.0 / 3.0, tag="BE3rep")
            SUB1rep = rep_row(vrow(sub1), c.AV, tag="SUB1rep")
            SIB1rep = rep_row(vrow(sib1), c.AV, tag="SIB1rep")
            SUW2rep = rep_row(vrow(suw2), c.AV, tag="SUW2rep")
            SIW2rep = rep_row(vrow(siw2), c.AV, tag="SIW2rep")
            CB1rep = rep_row(vrow(cb1), c.CH, tag="CB1rep")
            CW20rep = rep_row(cw2.ap()[:, 0:1].rearrange("a o -> o a"), c.CH, tag="CW20rep")
            CW21rep = rep_row(cw2.ap()[:, 1:2].rearrange("a o -> o a"), c.CH, tag="CW21rep")
            ATTNrep = [rep_row(attn.ap()[mp:mp + 1, :], HID, tag=f"ATTN{mp}")
                       for mp in range(c.n_mp)]

            # ---------- rotation constants (normalize r0 on device) ----------
            rcol = kpool.tile([HID, 1], F32, tag="rcol")
            nc.sync.dma_start(rcol[:], rvec.ap().rearrange("(p o) -> p o", o=1))
            idh = kpool.tile([HID, HID], F32, tag="idh")
            make_identity(nc, idh[:])
            Sp = kpool.tile([HID, HID], F32, tag="Sp")
            nc.vector.memset(Sp[:], 0.0)
            nc.vector.tensor_copy(Sp[:, 1:HID], idh[:, 0:HID - 1])
            Sm = kpool.tile([HID, HID], F32, tag="Sm")
            nc.vector.memset(Sm[:], 0.0)
            nc.vector.tensor_copy(Sm[:, 0:HID - 1], idh[:, 1:HID])
            pidx = kpool.tile([HID, 1], I32, tag="pidx")
            nc.gpsimd.iota(pidx[:], pattern=[[0, 1]], base=0, channel_multiplier=1)
            podd_i = kpool.tile([HID, 1], I32, tag="podd_i")
            nc.vector.tensor_scalar(podd_i[:], pidx[:], 1, None, ALU.bitwise_and)
            podd = kpool.tile([HID, 1], F32, tag="podd")
            nc.vector.tensor_copy(podd[:], podd_i[:])
            peven = kpool.tile([HID, 1], F32, tag="peven")
            nc.vector.tensor_scalar(peven[:], podd[:], -1.0, -1.0, ALU.add, ALU.mult)
            Spe = kpool.tile([HID, HID], F32, tag="Spe")
            nc.vector.tensor_scalar_mul(Spe[:], Sp[:], peven[:])
            Smo = kpool.tile([HID, HID], F32, tag="Smo")
            nc.vector.tensor_scalar_mul(Smo[:], Sm[:], podd[:])
            Ie = kpool.tile([HID, HID], F32, tag="Ie")
            nc.vector.tensor_scalar_mul(Ie[:], idh[:], peven[:])
            Io = kpool.tile([HID, HID], F32, tag="Io")
            nc.vector.tensor_scalar_mul(Io[:], idh[:], podd[:])
            M2 = kpool.tile([HID, HID], F32, tag="M2")
            nc.vector.tensor_tensor(M2[:], idh[:], Spe[:], ALU.add)
            nc.vector.tensor_tensor(M2[:], M2[:], Smo[:], ALU.add)
            Me = kpool.tile([HID, HID], F32, tag="Me")
            nc.vector.tensor_tensor(Me[:], Ie[:], Spe[:], ALU.add)
            Mo = kpool.tile([HID, HID], F32, tag="Mo")
            nc.vector.tensor_tensor(Mo[:], Io[:], Smo[:], ALU.add)
            sqc = kpool.tile([HID, 1], F32, tag="sqc")
            nc.vector.tensor_tensor(sqc[:], rcol[:], rcol[:], ALU.mult)
            n2 = pkpool.tile([HID, 1], F32, space=PSUM, tag="n2")
            nc.tensor.matmul(out=n2[:], lhsT=M2[:], rhs=sqc[:], start=True, stop=True)
            nrm = kpool.tile([HID, 1], F32, tag="nrm")
            nc.scalar.activation(nrm[:], n2[:], AF.Sqrt)
            invn = kpool.tile([HID, 1], F32, tag="invn")
            nc.vector.reciprocal(invn[:], nrm[:])
            rn = kpool.tile([HID, 1], F32, tag="rn")
            nc.vector.tensor_scalar_mul(rn[:], rcol[:], invn[:])
            cr2 = pkpool.tile([HID, 1], F32, space=PSUM, tag="cr2")
            nc.tensor.matmul(out=cr2[:], lhsT=Me[:], rhs=rn[:], start=True, stop=True)
            ci2 = pkpool.tile([HID, 1], F32, space=PSUM, tag="ci2")
            nc.tensor.matmul(out=ci2[:], lhsT=Mo[:], rhs=rn[:], start=True, stop=True)
            cr2s = kpool.tile([HID, 1], F32, tag="cr2s")
            nc.vector.tensor_copy(cr2s[:], cr2[:])
            ci2s = kpool.tile([HID, 1], F32, tag="ci2s")
            nc.vector.tensor_copy(ci2s[:], ci2[:])
            crrow_ps = pkpool.tile([1, HID], F32, space=PSUM, tag="crrow_ps")
            nc.tensor.matmul(out=crrow_ps[:], lhsT=cr2s[:], rhs=idh[:], start=True, stop=True)
            crrow = kpool.tile([1, HID], F32, tag="crrow")
            nc.vector.tensor_copy(crrow[:], crrow_ps[:])
            cirow_ps = pkpool.tile([1, HID], F32, space=PSUM, tag="cirow_ps")
            nc.tensor.matmul(out=cirow_ps[:], lhsT=ci2s[:], rhs=idh[:], start=True, stop=True)
            cirow = kpool.tile([1, HID], F32, tag="cirow")
            nc.vector.tensor_copy(cirow[:], cirow_ps[:])
            fidx = kpool.tile([1, HID], I32, tag="fidx")
            nc.gpsimd.iota(fidx[:], pattern=[[1, HID]], base=0, channel_multiplier=0)
            fodd_i = kpool.tile([1, HID], I32, tag="fodd_i")
            nc.vector.tensor_scalar(fodd_i[:], fidx[:], 1, None, ALU.bitwise_and)
            fsign = kpool.tile([1, HID], F32, tag="fsign")
            nc.vector.tensor_copy(fsign[:], fodd_i[:])
            nc.vector.tensor_scalar(fsign[:], fsign[:], -2.0, 1.0, ALU.mult, ALU.add)
            c2urow = kpool.tile([1, HID], F32, tag="c2urow")
            nc.vector.tensor_tensor(c2urow[:], cirow[:], fsign[:], ALU.mult)
            c2irow = kpool.tile([1, HID], F32, tag="c2irow")
            nc.vector.tensor_scalar_mul(c2irow[:], c2urow[:], -1.0)

            def rep_from_row(row, n, tag):
                ps = pkpool.tile([128, 512], F32, space=PSUM, tag="reppsum")
                nc.tensor.matmul(out=ps[:, :n], lhsT=ones1[:], rhs=row[:],
                                 start=True, stop=True)
                rep = kpool.tile([128, n], F32, tag=tag)
                nc.vector.tensor_copy(rep[:], ps[:, :n])
                return rep

            C1rep = rep_from_row(crrow, HID, "C1rep")
            C2rep = [rep_from_row(c2urow, HID, "C2urep"),
                     rep_from_row(c2irow, HID, "C2irep")]

            pwsb = kpool.tile([128, c.kF, HID], F32, tag="pwsb")
            nc.sync.dma_start(pwsb[:], pw.ap().rearrange("(a p) c -> p a c", p=128))
            w2sb = kpool.tile([HID, HID], F32, tag="w2sb")
            nc.sync.dma_start(w2sb[:], w2.ap())
            suw1sb = kpool.tile([HID, c.AV], F32, tag="suw1sb")
            nc.sync.dma_start(suw1sb[:], suw1.ap())
            siw1sb = kpool.tile([HID, c.AV], F32, tag="siw1sb")
            nc.sync.dma_start(siw1sb[:], siw1.ap())
            cw1sb = kpool.tile([HID, c.CH], F32, tag="cw1sb")
            nc.sync.dma_start(cw1sb[:], cw1.ap())

            pk_ctx.__exit__(None, None, None)

            # ---------- tower ----------
            tower_t = dpool.tile([c.nodes_core, HID], F32, tag="tower")
            table_t = dpool.tile([c.n_nodes, HID], F32, tag="table")
            with (
                tc.tile_pool(name="tw_x", bufs=2) as xpool,
                tc.tile_pool(name="tw_ps", bufs=1, space="PSUM") as tpspool,
                tc.tile_pool(name="tw_s", bufs=3) as tspool,
            ):
                for j in range(c.node_tiles):
                    xt = xpool.tile([128, c.F0], F32, tag="xt")
                    nc.sync.dma_start(xt[:], feats.ap()[j * 128:(j + 1) * 128, :])
                    xT = xpool.tile([128, c.kF, 128], F32, tag="xT")
                    for kk in range(c.kF):
                        pst = tpspool.tile([128, 128], F32, space=PSUM, tag="pst")
                        nc.tensor.transpose(pst[:], xt[:, kk * 128:(kk + 1) * 128], id128[:])
                        nc.vector.tensor_copy(xT[:, kk, :], pst[:])
                    z = tpspool.tile([128, HID], F32, space=PSUM, tag="z")
                    for kk in range(c.kF):
                        nc.tensor.matmul(out=z[:], lhsT=xT[:, kk, :], rhs=pwsb[:, kk, :],
                                         start=(kk == 0), stop=(kk == c.kF - 1))
                    zb = tspool.tile([128, HID], F32, tag="zb")
                    nc.vector.tensor_tensor(zb[:], z[:], PBrep[:], ALU.add)
                    h = tspool.tile([128, HID], F32, tag="h")
                    nc.scalar.activation(h[:], zb[:], AF.Gelu if c.gelu else AF.Tanh)
                    hT_ps = tpspool.tile([HID, 128], F32, space=PSUM, tag="hT_ps")
                    nc.tensor.transpose(hT_ps[:], h[:], id128[:])
                    hT = tspool.tile([HID, 128], F32, tag="hT")
                    nc.vector.tensor_copy(hT[:], hT_ps[:])
                    y = tpspool.tile([128, HID], F32, space=PSUM, tag="y")
                    nc.tensor.matmul(out=y[:], lhsT=hT[:], rhs=w2sb[:], start=True, stop=True)
                    ys = tspool.tile([128, HID], F32, tag="ys")
                    nc.vector.tensor_tensor(ys[:], y[:], B2rep[:], ALU.add)
                    nc.vector.tensor_tensor(ys[:], ys[:], zb[:], ALU.add)
                    mu = tspool.tile([128, 1], F32, tag="mu")
                    nc.vector.tensor_reduce(mu[:], ys[:], mybir.AxisListType.X, ALU.add)
                    nc.vector.tensor_scalar_mul(mu[:], mu[:], 1.0 / HID)
                    yc = tspool.tile([128, HID], F32, tag="yc")
                    nc.vector.tensor_scalar(yc[:], ys[:], mu[:], None, ALU.subtract)
                    sq = tspool.tile([128, HID], F32, tag="sq")
                    nc.vector.tensor_tensor(sq[:], yc[:], yc[:], ALU.mult)
                    vv = tspool.tile([128, 1], F32, tag="vv")
                    nc.vector.tensor_reduce(vv[:], sq[:], mybir.AxisListType.X, ALU.add)
                    sdv = tspool.tile([128, 1], F32, tag="sdv")
                    nc.scalar.activation(sdv[:], vv[:], AF.Sqrt, bias=epscol[:], scale=1.0 / HID)
                    inv = tspool.tile([128, 1], F32, tag="inv")
                    nc.vector.reciprocal(inv[:], sdv[:])
                    nc.vector.tensor_scalar_mul(yc[:], yc[:], inv[:])
                    tbl = tspool.tile([128, HID], F32, tag="tbl")
                    nc.vector.tensor_tensor(tbl[:], yc[:], G3rep[:], ALU.mult)
                    nc.vector.tensor_tensor(tbl[:], tbl[:], BE3rep[:], ALU.add)
                    rows = min(128, c.nodes_core - j * 128)
                    nc.sync.dma_start(tower_t[j * 128:j * 128 + rows, :], tbl[:rows, :])

            nc.gpsimd.collective_compute(
                "AllGather", ALU.bypass,
                replica_groups=[list(range(c.n_cores))],
                ins=[tower_t.opt()], outs=[table_t.opt()],
            )

            # ---------- metapaths: gather, rotate, logits, dump rows ----------
            rowdumps = []
            with (
                tc.tile_pool(name="mp_idx", bufs=2) as ipool,
                tc.tile_pool(name="mp_ed", bufs=2) as edpool,
                tc.tile_pool(name="mp_row", bufs=2) as rowpool,
                tc.tile_pool(name="mp_tmp", bufs=2) as mtpool,
            ):
                for mp in range(c.n_mp):
                    side = 0 if mp < 2 else 1
                    calls = _gather_calls(c.tiles_per_class[mp], c.Tc, c.n_chunks)
                    emi_sb = ipool.tile([128, 3, T8], I16, tag="emi_sb")
                    nc.sync.dma_start(
                        emi_sb[:],
                        emi16.ap()[mp * 3 * 128:(mp + 1) * 3 * 128, :]
                        .rearrange("(l p) s -> p l s", p=128))
                    rowd = dpool.tile([c.E_loc + 128, 128], F32, tag=f"rowd{mp}")
                    rowdumps.append(rowd)
                    nc.sync.dma_start(rowd[c.E_loc:c.E_loc + 128, :], zerot[:])
                    for ch in range(c.n_chunks):
                        ed = edpool.tile([128, 3, c.Tc, HID], F32, tag="ed")
                        for l in range(3):
                            for (toff, nt, hi) in calls[ch][l]:
                                src = (table_t[c.LO:c.n_nodes, :] if hi
                                       else table_t[0:c.LO, :])
                                nc.gpsimd.dma_gather(
                                    out_ap=ed[:, l, toff:toff + nt, :],
                                    in_ap=src,
                                    idxs_ap=emi_sb[:, l,
                                                   (ch * c.Tc + toff) * 8:
                                                   (ch * c.Tc + toff + nt) * 8],
                                    num_idxs=nt * 128, num_idxs_reg=nt * 128,
                                    elem_size=HID, single_packet=False)
                        row = rowpool.tile([128, c.Tc, 128], F32, tag="row")
                        nc.vector.memset(row[:, :, HID + H:128], 0.0)
                        eftv = row[:, :, 0:HID]
                        ed0, ed1, ed2 = ed[:, 0], ed[:, 1], ed[:, 2]
                        nc.vector.tensor_tensor(eftv, ed0, ed2, ALU.add)
                        ta = mtpool.tile([128, c.Tc, HID], F32, tag="ta")
                        c1b = _ap_with(C1rep[:], 0, [[0, c.Tc], [1, HID]])
                        nc.vector.tensor_tensor(ta[:], ed1, c1b, ALU.mult)
                        tb = mtpool.tile([128, c.Tc, HID], F32, tag="tb")
                        ed1s = _ap_with(ed1, 1, [list(ed1.ap[1]), [2, NPAIR], [-1, 2]])
                        c2b = _ap_with(C2rep[side][:], 0, [[0, c.Tc], [1, HID]])
                        nc.vector.tensor_tensor(tb[:], ed1s, c2b, ALU.mult)
                        nc.vector.tensor_tensor(eftv, eftv, ta[:], ALU.add)
                        nc.vector.tensor_tensor(eftv, eftv, tb[:], ALU.add)
                        t5 = mtpool.tile([128, c.Tc, HID], F32, tag="t5")
                        atb = _ap_with(ATTNrep[mp][:], 0, [[0, c.Tc], [1, HID]])
                        nc.vector.tensor_tensor(t5[:], eftv, atb, ALU.mult)
                        ep = mtpool.tile([128, c.Tc, H], F32, tag="ep")
                        nc.vector.tensor_reduce(
                            ep[:], t5[:].rearrange("p t (h d) -> p t h d", d=D),
                            mybir.AxisListType.X, ALU.add)
                        epl = mtpool.tile([128, c.Tc, H], F32, tag="epl")
                        nc.vector.tensor_scalar_mul(epl[:], ep[:], 0.01)
                        nc.vector.tensor_tensor(epl[:], epl[:], ep[:], ALU.max)
                        av = row[:, :, HID:HID + H]
                        nc.scalar.activation(av, epl[:], AF.Exp)
                        avb = _ap_with(row[:], HID, [[128, c.Tc], [1, H], [0, D]])
                        nc.vector.tensor_tensor(eftv, eftv, avb, ALU.mult)
                        if c.debug and mp == 0 and ch == 0:
                            nc.sync.dma_start(
                                dbg_ed.ap(), ed[:].rearrange("p l t e -> p (l t e)"))
                            nc.sync.dma_start(
                                dbg_row.ap(), row[:].rearrange("p t e -> p (t e)"))
                        # row (t, p) -> DRAM row p*T + t: per-partition contiguous
                        dst = bass.AP(
                            rowd[:].tensor,
                            rowd[:].offset + ch * c.Tc * 128,
                            [[c.T * 128, 128], [128, c.Tc], [1, 128]])
                        nc.sync.dma_start(dst, row[:])

            # ---------- head ----------
            with (
                tc.tile_pool(name="hd_s", bufs=3) as hpool,
                tc.tile_pool(name="hd_ps", bufs=1, space="PSUM") as hpspool,
                tc.tile_pool(name="hd_keep", bufs=1) as keep,
            ):
                outs_all = keep.tile([128, c.n_mp, c.b_tiles, HID], F32, tag="outs_all")
                acc4 = keep.tile([1, c.n_mp], F32, tag="acc4")
                nc.vector.memset(acc4[:], 0.0)
                for mp in range(c.n_mp):
                    w1sb = suw1sb if mp < 2 else siw1sb
                    b1rep = SUB1rep if mp < 2 else SIB1rep
                    w2rep = SUW2rep if mp < 2 else SIW2rep
                    gix = keep.tile([128, c.b_tiles, c.KP_max * 8], I16, tag="gix")
                    nc.sync.dma_start(
                        gix[:],
                        gidx16.ap()[mp * c.b_tiles * 128:(mp + 1) * c.b_tiles * 128, :]
                        .rearrange("(b p) s -> p b s", p=128))
                    for bt in range(c.b_tiles):
                        kp = int(c.KP[mp][bt])
                        pg = hpool.tile([128, c.KP_max, 128], F32, tag="pg")
                        nc.gpsimd.dma_gather(
                            out_ap=pg[:, 0:kp, :], in_ap=rowdumps[mp][:],
                            idxs_ap=gix[:, bt, 0:kp * 8],
                            num_idxs=kp * 128, num_idxs_reg=kp * 128,
                            elem_size=128, single_packet=False)
                        red = hpool.tile([128, 72], F32, tag="red")
                        r_in = _ap_with(pg[:], 0, [[1, 72], [128, kp]])
                        nc.vector.tensor_reduce(red[:], r_in, mybir.AxisListType.X, ALU.add)
                        if c.debug and mp == 0 and bt == 0:
                            nc.sync.dma_start(dbg_acc.ap(), red[:])
                        den = hpool.tile([128, H], F32, tag="den")
                        nc.vector.tensor_scalar_add(den[:], red[:, HID:HID + H], 1e-9)
                        dinv = hpool.tile([128, H], F32, tag="dinv")
                        nc.vector.reciprocal(dinv[:], den[:])
                        ret = hpool.tile([128, HID], F32, tag="ret")
                        dinvb = _ap_with(dinv[:], 0, [[1, H], [0, D]])
                        nc.vector.tensor_tensor(ret[:], red[:, 0:HID], dinvb, ALU.mult)
                        neg = hpool.tile([128, HID], F32, tag="neg")
                        nc.vector.tensor_scalar_min(neg[:], ret[:], 0.0)
                        en = hpool.tile([128, HID], F32, tag="en")
                        nc.scalar.activation(en[:], neg[:], AF.Exp)
                        o = outs_all[:, mp, bt, :]
                        nc.vector.tensor_scalar_max(ret[:], ret[:], 0.0)
                        nc.vector.tensor_scalar_add(en[:], en[:], -1.0)
                        nc.vector.tensor_tensor(o, ret[:], en[:], ALU.add)
                        oT_ps = hpspool.tile([HID, 128], F32, space=PSUM, tag="oT_ps")
                        nc.tensor.transpose(oT_ps[:], o, id128[:])
                        oT = hpool.tile([HID, 128], F32, tag="oT")
                        nc.vector.tensor_copy(oT[:], oT_ps[:])
                        tt = hpspool.tile([128, c.AV], F32, space=PSUM, tag="tt")
                        nc.tensor.matmul(out=tt[:], lhsT=oT[:], rhs=w1sb[:], start=True, stop=True)
                        th = hpool.tile([128, c.AV], F32, tag="th")
                        nc.vector.tensor_tensor(th[:], tt[:], b1rep[:], ALU.add)
                        nc.scalar.activation(th[:], th[:], AF.Tanh)
                        nc.vector.tensor_tensor(th[:], th[:], w2rep[:], ALU.mult)
                        rsum = hpool.tile([128, 1], F32, tag="rsum")
                        nc.vector.tensor_reduce(rsum[:], th[:], mybir.AxisListType.X, ALU.add)
                        sp = hpspool.tile([1, 1], F32, space=PSUM, tag="sp")
                        nc.tensor.matmul(out=sp[:], lhsT=rsum[:], rhs=onescol[:], start=True, stop=True)
                        nc.vector.tensor_tensor(acc4[:, mp:mp + 1], acc4[:, mp:mp + 1], sp[:], ALU.add)

                sin_t = dpool.tile([1, 128], F32, tag="sin")
                sout_t = dpool.tile([1, 128], F32, tag="sout")
                nc.sync.dma_start(sin_t[:], zerot[:1, :128])
                nc.sync.dma_start(sin_t[0:1, 0:c.n_mp], acc4[:])
                nc.gpsimd.collective_compute(
                    "AllReduce", ALU.add,
                    replica_groups=[list(range(c.n_cores))],
                    ins=[sin_t.opt()], outs=[sout_t.opt()],
                )
                s4 = hpool.tile([1, c.n_mp], F32, tag="s4")
                nc.sync.dma_start(s4[:], sout_t[0:1, 0:c.n_mp])
                e4 = hpool.tile([1, c.n_mp], F32, tag="e4")
                nc.scalar.activation(e4[:], s4[:], AF.Exp, scale=1.0 / c.B)
                beta = hpool.tile([1, c.n_mp], F32, tag="beta")
                for sd in range(2):
                    ssum = hpool.tile([1, 1], F32, tag="ssum")
                    nc.vector.tensor_reduce(ssum[:], e4[:, 2 * sd:2 * sd + 2],
                                            mybir.AxisListType.X, ALU.add)
                    sinv = hpool.tile([1, 1], F32, tag="sinv")
                    nc.vector.reciprocal(sinv[:], ssum[:])
                    nc.vector.tensor_scalar_mul(beta[:, 2 * sd:2 * sd + 2],
                                                e4[:, 2 * sd:2 * sd + 2], sinv[:])
                if c.debug:
                    dsem = hpool.tile([1, 12], F32, tag="dsem")
                    nc.vector.tensor_copy(dsem[:, 0:4], acc4[:])
                    nc.vector.tensor_copy(dsem[:, 4:8], s4[:])
                    nc.vector.tensor_copy(dsem[:, 8:12], beta[:])
                    nc.sync.dma_start(dbg_sem.ap(), dsem[:])
                bc_ps = hpspool.tile([128, c.n_mp], F32, space=PSUM, tag="bc_ps")
                nc.tensor.matmul(out=bc_ps[:], lhsT=ones1[:], rhs=beta[:], start=True, stop=True)
                bcol = keep.tile([128, c.n_mp], F32, tag="bcol")
                nc.vector.tensor_copy(bcol[:], bc_ps[:])

                for bt in range(c.b_tiles):
                    hu = hpool.tile([128, HID], F32, tag="hu")
                    hi_ = hpool.tile([128, HID], F32, tag="hi_")
                    t0 = hpool.tile([128, HID], F32, tag="t0")
                    nc.vector.tensor_scalar_mul(hu[:], outs_all[:, 0, bt, :], bcol[:, 0:1])
                    nc.vector.tensor_scalar_mul(t0[:], outs_all[:, 1, bt, :], bcol[:, 1:2])
                    nc.vector.tensor_tensor(hu[:], hu[:], t0[:], ALU.add)
                    nc.vector.tensor_scalar_mul(hi_[:], outs_all[:, 2, bt, :], bcol[:, 2:3])
                    nc.vector.tensor_scalar_mul(t0[:], outs_all[:, 3, bt, :], bcol[:, 3:4])
                    nc.vector.tensor_tensor(hi_[:], hi_[:], t0[:], ALU.add)
                    xx = hpool.tile([128, HID], F32, tag="xx")
                    nc.vector.tensor_tensor(xx[:], hu[:], hi_[:], ALU.mult)
                    xT_ps = hpspool.tile([HID, 128], F32, space=PSUM, tag="xT_ps")
                    nc.tensor.transpose(xT_ps[:], xx[:], id128[:])
                    xT = hpool.tile([HID, 128], F32, tag="xT")
                    nc.vector.tensor_copy(xT[:], xT_ps[:])
                    yy = hpspool.tile([128, c.CH], F32, space=PSUM, tag="yy")
                    nc.tensor.matmul(out=yy[:], lhsT=xT[:], rhs=cw1sb[:], start=True, stop=True)
                    ya = hpool.tile([128, c.CH], F32, tag="ya")
                    nc.vector.tensor_tensor(ya[:], yy[:], CB1rep[:], ALU.add)
                    nc.scalar.activation(ya[:], ya[:], AF.Relu)
                    l0t = hpool.tile([128, c.CH], F32, tag="l0t")
                    nc.vector.tensor_tensor(l0t[:], ya[:], CW20rep[:], ALU.mult)
                    l0 = hpool.tile([128, 1], F32, tag="l0")
                    nc.vector.tensor_reduce(l0[:], l0t[:], mybir.AxisListType.X, ALU.add)
                    nc.vector.tensor_tensor(l0t[:], ya[:], CW21rep[:], ALU.mult)
                    l1 = hpool.tile([128, 1], F32, tag="l1")
                    nc.vector.tensor_reduce(l1[:], l0t[:], mybir.AxisListType.X, ALU.add)
                    dl = hpool.tile([128, 1], F32, tag="dl")
                    ot = hpool.tile([128, 2], F32, tag="ot")
                    nc.vector.tensor_tensor(dl[:], l0[:], l1[:], ALU.subtract)
                    nc.scalar.activation(ot[:, 0:1], dl[:], AF.Sigmoid)
                    nc.vector.tensor_tensor(dl[:], l1[:], l0[:], ALU.subtract)
                    nc.scalar.activation(ot[:, 1:2], dl[:], AF.Sigmoid)
                    nc.sync.dma_start(outd.ap()[bt * 128:(bt + 1) * 128, :], ot[:])

    nc.compile()
    return nc


# ---------------------------------------------------------------------------
# host side: sharding / packing
# ---------------------------------------------------------------------------

def _mp_arrays(inputs, mp):
    if mp < 2:
        return np.asarray(inputs["emi_user"][mp]), np.asarray(inputs["tgt_user"][mp])
    return np.asarray(inputs["emi_item"][mp - 2]), np.asarray(inputs["tgt_item"][mp - 2])


def make_plan(inputs, cfg: Cfg):
    c = cfg
    tpc = np.zeros((c.n_mp, 8), np.int64)
    for mp in range(c.n_mp):
        emi, tgt = _mp_arrays(inputs, mp)
        for k in range(c.n_cores):
            sel = (tgt >= k * c.B_loc) & (tgt < (k + 1) * c.B_loc)
            e = emi[sel]
            cls = ((e[:, 0] >= c.LO).astype(int) + 2 * (e[:, 1] >= c.LO) +
                   4 * (e[:, 2] >= c.LO))
            cnt = np.bincount(cls, minlength=8)
            tpc[mp] = np.maximum(tpc[mp], (cnt + 127) // 128)
    T = int(tpc.sum(1).max())
    T = ((T + c.Tc - 1) // c.Tc) * c.Tc
    for mp in range(c.n_mp):
        tpc[mp, 7] += T - tpc[mp].sum()
    return tpc, T


def _wrap16(vals):
    """[N] values (N % 16 == 0) -> [128, N/16] int16, q7 wrapped layout."""
    v = np.asarray(vals).astype(np.int16).reshape(-1, 16)
    return np.ascontiguousarray(np.tile(v.T, (8, 1)))


def _pack_metapath(emi, tgt, k, c: Cfg, tpc_mp):
    """Pack one (metapath, core) shard grouped by class.

    Returns (idx16 [3,128,T*8], tloc [E_loc] local target per position,
    -1 for padding)."""
    lo, hi = k * c.B_loc, (k + 1) * c.B_loc
    sel = np.nonzero((tgt >= lo) & (tgt < hi))[0]
    e_all, t_all = emi[sel], tgt[sel] - lo
    cls_all = ((e_all[:, 0] >= c.LO).astype(int) + 2 * (e_all[:, 1] >= c.LO) +
               4 * (e_all[:, 2] >= c.LO))
    E = c.E_loc
    emi_sh = np.zeros((E, 3), np.int64)
    tloc = np.full((E,), -1, np.int64)
    tpos = 0
    for cl in range(8):
        ntiles = int(tpc_mp[cl])
        if ntiles == 0:
            continue
        seg = np.nonzero(cls_all == cl)[0]
        assert seg.size <= ntiles * 128
        base = tpos * 128
        emi_sh[base:base + seg.size] = e_all[seg]
        dummy = np.array([c.LO if (cl >> l) & 1 else 0 for l in range(3)], np.int64)
        emi_sh[base + seg.size:base + ntiles * 128] = dummy
        tloc[base:base + seg.size] = t_all[seg]
        tpos += ntiles
    assert tpos == c.T
    idx16 = []
    for l in range(3):
        v = emi_sh[:, l].copy()
        v[v >= c.LO] -= c.LO
        idx16.append(_wrap16(v))
    return np.stack(idx16), tloc


def prepare(inputs, cfg: Cfg):
    """Plan, pack all shards, choose per-b-tile gather widths.

    Returns (in_maps, perms); perms[k] is core k's target permutation."""
    c = cfg
    tpc, T = make_plan(inputs, cfg)
    c.tiles_per_class = tpc
    c.T = T
    packs = {}
    counts = np.zeros((c.n_mp, c.n_cores, c.B_loc), np.int64)
    for mp in range(c.n_mp):
        emi, tgt = _mp_arrays(inputs, mp)
        for k in range(c.n_cores):
            et, tloc = _pack_metapath(emi, tgt, k, c, tpc[mp])
            packs[(mp, k)] = (et, tloc)
            counts[mp, k] = np.bincount(tloc[tloc >= 0], minlength=c.B_loc)
    perms = [np.argsort(-counts[:, k].sum(0), kind="stable")
             for k in range(c.n_cores)]
    KP = np.ones((c.n_mp, c.b_tiles), np.int64)
    for mp in range(c.n_mp):
        for k in range(c.n_cores):
            sc = counts[mp, k][perms[k]].reshape(c.b_tiles, 128).max(1)
            KP[mp] = np.maximum(KP[mp], sc)
    c.KP = KP

    f0, f1 = np.asarray(inputs["feats0"]), np.asarray(inputs["feats1"])
    feats_all = np.concatenate([f0, f1], axis=0)
    attn4 = np.stack([np.asarray(inputs["attn_user"][p]).reshape(-1) for p in range(2)] +
                     [np.asarray(inputs["attn_item"][p]).reshape(-1) for p in range(2)])
    rv = np.asarray(inputs["r_vec"])[0].reshape(-1).astype(np.float32)

    in_maps = []
    for k in range(c.n_cores):
        m = {}
        lo_n = k * c.nodes_core
        fs = feats_all[lo_n:lo_n + c.nodes_core]
        pad = c.node_tiles * 128 - c.nodes_core
        if pad:
            fs = np.concatenate([fs, np.zeros((pad, c.F0), np.float32)], axis=0)
        m["feats"] = np.ascontiguousarray(fs, np.float32)
        tw = "0" if lo_n < f0.shape[0] else "1"
        for nm in ("pw", "pb", "w2", "b2", "g", "be"):
            m[nm] = np.asarray(inputs[f"tower{tw}_{nm}"], np.float32)
        m["rvec"] = rv
        m["attn"] = attn4.astype(np.float32)
        blmap = np.empty(c.B_loc, np.int64)
        blmap[perms[k]] = np.arange(c.B_loc)
        emi_l, gix_l = [], []
        for mp in range(c.n_mp):
            et, tloc = packs[(mp, k)]
            emi_l.append(et)
            # vectorized per-target rank + scatter into the gather-index grid
            order = np.argsort(tloc, kind="stable")
            ts = tloc[order]
            valid = ts >= 0
            ov, ts = order[valid], ts[valid]
            rk = np.arange(ov.size) - np.searchsorted(ts, ts, side="left")
            slots = (ov % 128) * c.T + ov // 128
            bl = blmap[ts]
            gv = np.full((c.b_tiles, c.KP_max, 128), c.E_loc, np.int64)
            gv.reshape(-1)[(bl // 128) * (c.KP_max * 128) + rk * 128 + bl % 128] = slots
            gix_l.append(np.stack([_wrap16(gv[bt].reshape(-1))
                                   for bt in range(c.b_tiles)]))
        m["emi16"] = np.concatenate(emi_l).reshape(c.n_mp * 3 * 128, c.T * 8)
        m["gidx16"] = np.concatenate(gix_l).reshape(
            c.n_mp * c.b_tiles * 128, c.KP_max * 8)
        for nm in ("su_w1", "su_b1", "su_w2", "si_w1", "si_b1", "si_w2",
                   "cw1", "cb1", "cw2"):
            m[nm.replace("_", "")] = np.asarray(inputs[nm], np.float32)
        in_maps.append(m)
    return in_maps, perms


# ---------------------------------------------------------------------------
# PJRT SPMD runner (axon path)
# ---------------------------------------------------------------------------


class SpmdRunner:
    def __init__(self, nc, n_cores: int):
        import jax
        from jax.sharding import Mesh, PartitionSpec, NamedSharding
        from jax.experimental.shard_map import shard_map
        from concourse.bass2jax import (
            _bass_exec_p, install_neuronx_cc_hook, partition_id_tensor)

        self.jax = jax
        install_neuronx_cc_hook()
        self.nc = nc
        self.n_cores = n_cores
        partition_name = nc.partition_id_tensor.name if nc.partition_id_tensor else None
        in_names, out_names, out_avals, zero_outs = [], [], [], []
        for alloc in nc.m.functions[0].allocations:
            if not isinstance(alloc, mybir.MemoryLocationSet):
                continue
            name = alloc.memorylocations[0].name
            if alloc.kind == "ExternalInput":
                if name != partition_name:
                    in_names.append(name)
            elif alloc.kind == "ExternalOutput":
                out_names.append(name)
                shape = tuple(alloc.tensor_shape)
                dtype = mybir.dt.np(alloc.dtype)
                out_avals.append(jax.core.ShapedArray(shape, dtype))
                zero_outs.append(np.zeros(shape, dtype))
        self.dbg_name = nc.dbg_addr.name if nc.dbg_addr is not None else None
        n_params = len(in_names)
        in_names = in_names + out_names
        if partition_name is not None:
            in_names.append(partition_name)
        self.in_names, self.out_names = in_names, out_names
        self.n_params, self.out_avals, self.zero_outs = n_params, out_avals, zero_outs

        def _body(*args):
            operands = list(args)
            if partition_name is not None:
                operands.append(partition_id_tensor())
            outs = _bass_exec_p.bind(
                *operands,
                out_avals=tuple(out_avals),
                in_names=tuple(in_names),
                out_names=tuple(out_names),
                lowering_input_output_aliases=(),
                sim_require_finite=True,
                sim_require_nnan=True,
                nc=nc,
            )
            return tuple(outs)

        devices = jax.devices()[:n_cores]
        assert len(devices) == n_cores
        self.mesh = Mesh(np.asarray(devices), ("core",))
        donate = tuple(range(n_params, n_params + len(out_names)))
        in_specs = (PartitionSpec("core"),) * (n_params + len(out_names))
        out_specs = (PartitionSpec("core"),) * len(out_names)
        self.sharded = jax.jit(
            shard_map(_body, mesh=self.mesh, in_specs=in_specs,
                      out_specs=out_specs, check_rep=False),
            donate_argnums=donate, keep_unused=True)
        self.sharding = NamedSharding(self.mesh, PartitionSpec("core"))

    def stage_inputs(self, in_maps):
        jax = self.jax
        if self.dbg_name is not None:
            in_maps = [{**m, self.dbg_name: np.zeros((1, 2), np.uint32)}
                       for m in in_maps]
        staged = []
        for i in range(self.n_params):
            name = self.in_names[i]
            arr = np.concatenate([np.asarray(m[name]) for m in in_maps], axis=0)
            staged.append(jax.device_put(arr, self.sharding))
        jax.block_until_ready(staged)
        self.staged = staged

    def _zeros(self):
        jax = self.jax
        zs = [jax.device_put(
            np.zeros((self.n_cores * z.shape[0], *z.shape[1:]), z.dtype),
            self.sharding) for z in self.zero_outs]
        jax.block_until_ready(zs)
        return zs

    def run(self):
        jax = self.jax
        outs = self.sharded(*self.staged, *self._zeros())
        jax.block_until_ready(outs)
        return [
            {name: np.asarray(outs[i]).reshape(self.n_cores, *self.out_avals[i].shape)[k]
             for i, name in enumerate(self.out_names)}
            for k in range(self.n_cores)
        ]

    def bench(self, iters=20, warmup=3):
        import time
        jax = self.jax
        times = []
        for it in range(warmup + iters):
            zs = self._zeros()
            t0 = time.perf_counter()
            outs = self.sharded(*self.staged, *zs)
            jax.block_until_ready(outs)
            dt = time.perf_counter() - t0
            if it >= warmup:
                times.append(dt)
            del outs
        times = np.array(times)
        return {"min_s": float(times.min()), "med_s": float(np.median(times)),
                "mean_s": float(times.mean()), "n": iters}


_CACHE = {}


def kernel(**inputs) -> np.ndarray:
    cfg = Cfg()
    in_maps, perms = prepare(inputs, cfg)
    key = (cfg.T, cfg.KP.tobytes(), cfg.tiles_per_class.tobytes())
    if key not in _CACHE:
        nc = build_program(cfg)
        _CACHE[key] = (nc, SpmdRunner(nc, cfg.n_cores))
    nc, runner = _CACHE[key]
    runner.stage_inputs(in_maps)
    res = runner.run()
    out = np.empty((cfg.B, 2), np.float32)
    for k in range(cfg.n_cores):
        inv = np.empty(cfg.B_loc, np.int64)
        inv[perms[k]] = np.arange(cfg.B_loc)
        out[k * cfg.B_loc:(k + 1) * cfg.B_loc] = res[k]["out"][inv]
    return out

